# revision 1
# baseline (speedup 1.0000x reference)
"""Trainium2 Bass kernel for a dense transformer block (B=4, T=2048, D=1024, H=16).

Sharding: 8 cores = 4 batches x 2 query-parity groups. Each core computes
LN1+QKV (K/V for the full sequence, all heads) for its batch, then causal
attention + proj + FFN for its half of the query blocks only. Query blocks are
assigned in a zigzag so causal attention work is balanced across the two cores
of a batch:
  parity 0 -> blocks [15,12,11,8,7,4,3,0]   (causal lengths 16,13,12,9,8,5,4,1)
  parity 1 -> blocks [14,13,10,9,6,5,2,1]   (causal lengths 15,14,11,10,7,6,3,2)
Slot j on every core processes NKMAX[j] = [16,14,12,10,8,6,4,2] key blocks, so
the SPMD program is identical on all 8 cores; the causal mask for the last two
key blocks of each slot is an input (differs by parity). The host permutes each
core's query tokens into slot order and inverse-permutes the output.

Matmul operands are bf16 (fp32 PE matmul is 4x slower); accumulation, softmax
denominators, layernorm stats and residuals are fp32. Softmax skips the
max-subtraction (scores are ~N(0,1) after the 1/8 scale; exp is exact softmax
mathematically and safe in fp32/bf16 range here), which lets the denominator
come from an appended ones-column in V during the PV matmul.
"""

import sys

try:
    import concourse  # noqa: F401
except ImportError:
    sys.path.insert(0, "/opt/trn_rl_repo")

import numpy as np
import ml_dtypes

import concourse.bass as bass
import concourse.bacc as bacc
import concourse.tile as tile
from concourse import mybir
from concourse.bass_utils import run_bass_kernel_spmd

F32 = mybir.dt.float32
BF16 = mybir.dt.bfloat16
AF = mybir.ActivationFunctionType
ALU = mybir.AluOpType
PSUM = bass.MemorySpace.PSUM

B, T, D = 4, 2048, 1024
H, HD = 16, 64
HID = 4 * D
EPS = 1e-5
N_CORES = 8
NB = T // 128  # 16 token blocks per batch
NQ = 8  # query slots per core
TQ = NQ * 128  # query tokens per core

QB = [
    [15, 12, 11, 8, 7, 4, 3, 0],
    [14, 13, 10, 9, 6, 5, 2, 1],
]
NKMAX = [16, 14, 12, 10, 8, 6, 4, 2]
MM_BUFS = 2
SC_BUFS = 2


def _perm(p):
    idx = []
    for qb in QB[p]:
        idx.extend(range(qb * 128, (qb + 1) * 128))
    return np.array(idx, dtype=np.int64)


def _masks(p):
    """[NQ, 128, 256] multiplicative mask for the last two key blocks per slot."""
    m = np.zeros((NQ, 128, 256), dtype=np.float32)
    tri = np.triu(np.ones((128, 128), np.float32))  # valid where k_local <= q_local
    for j in range(NQ):
        nk = QB[p][j] + 1
        nkm = NKMAX[j]
        for c in range(2):
            ki = nkm - 2 + c
            if ki < nk - 1:
                m[j, :, c * 128:(c + 1) * 128] = 1.0
            elif ki == nk - 1:
                m[j, :, c * 128:(c + 1) * 128] = tri
    return m


# ---------------------------------------------------------------------------
# Device program
# ---------------------------------------------------------------------------

def _layernorm_tile(nc, pool, x_ap, out_ap, eps_t):
    """LN (without gain/bias) of a [128, D] fp32 tile -> out_ap bf16.

    Stats on DVE (bn_stats); the normalize itself runs on the scalar engine
    as out = rstd*x + (-mean*rstd), keeping DVE free for the next tile.
    """
    stats = pool.tile([128, 2, nc.vector.BN_STATS_DIM], F32, tag="ln_stats")
    mv = pool.tile([128, nc.vector.BN_AGGR_DIM], F32, tag="ln_mv")
    nmr = pool.tile([128, 1], F32, tag="ln_nmr")
    xg = x_ap.rearrange("p (s d) -> p s d", s=2)
    for s in range(2):
        nc.vector.bn_stats(out=stats[:, s, :], in_=xg[:, s, :])
    nc.vector.bn_aggr(out=mv[:], in_=stats[:])
    mean = mv[:, 0:1]
    rstd = mv[:, 1:2]
    nc.scalar.activation(out=rstd, in_=rstd, func=AF.Sqrt, bias=eps_t[:], scale=1.0)
    nc.vector.reciprocal(out=rstd, in_=rstd)
    nc.vector.tensor_scalar(
        out=nmr[:], in0=mean, scalar1=rstd, scalar2=-1.0,
        op0=ALU.mult, op1=ALU.mult,
    )
    nc.scalar.activation(out=out_ap, in_=x_ap, func=AF.Identity,
                         bias=nmr[:], scale=rstd)


def build_program(niter=None, stop_after=None):
    nc = bacc.Bacc("TRN2", target_bir_lowering=False, debug=False,
                   num_devices=N_CORES)

    xf_d = nc.dram_tensor("x_full", [T, D], F32, kind="ExternalInput")
    xq_d = nc.dram_tensor("x_q", [TQ, D], F32, kind="ExternalInput")
    mask_d = nc.dram_tensor("mask", [128, NQ, 256], BF16, kind="ExternalInput")
    wqkv_d = nc.dram_tensor("wqkv", [128, 8, 3 * D], BF16, kind="ExternalInput")
    wproj_d = nc.dram_tensor("wproj", [128, 8, D], BF16, kind="ExternalInput")
    wff1_d = nc.dram_tensor("wff1", [128, 8, HID], BF16, kind="ExternalInput")
    wff2_d = nc.dram_tensor("wff2", [128, 32, D], BF16, kind="ExternalInput")
    bqk_d = nc.dram_tensor("bqk", [128, 16], F32, kind="ExternalInput")
    bv_d = nc.dram_tensor("bv", [D], F32, kind="ExternalInput")
    bproj_d = nc.dram_tensor("bproj", [D], F32, kind="ExternalInput")
    bff1_d = nc.dram_tensor("bff1", [128, 32], F32, kind="ExternalInput")
    bff2_d = nc.dram_tensor("bff2", [D], F32, kind="ExternalInput")
    g1_d = nc.dram_tensor("g1", [128, 8], F32, kind="ExternalInput")
    b1_d = nc.dram_tensor("b1", [128, 8], F32, kind="ExternalInput")
    g2_d = nc.dram_tensor("g2", [128, 8], F32, kind="ExternalInput")
    b2_d = nc.dram_tensor("b2", [128, 8], F32, kind="ExternalInput")
    y_d = nc.dram_tensor("y", [TQ, D], F32, kind="ExternalOutput")

    def bcast_row(dram_handle, n):
        ap = dram_handle.ap()
        return bass.AP(tensor=ap.tensor, offset=ap.offset, ap=[[0, 128], [1, n]])

    dram = dict(xf=xf_d, xq=xq_d, wqkv=wqkv_d, wproj=wproj_d,
                wff1=wff1_d, wff2=wff2_d, y=y_d)

    with tile.TileContext(nc) as tc:
        with tc.tile_pool(name="singles", bufs=1) as singles:
            ident = singles.tile([128, 128], BF16)
            from concourse.masks import make_identity
            make_identity(nc, ident[:])
            nc._ident = ident  # stashed for _transpose_block

            st = {}
            st["eps"] = singles.tile([128, 1], F32, name="eps", tag="eps")
            nc.vector.memset(st["eps"][:], EPS)
            for nm, dt, shape, src in (
                ("g1", F32, [128, 8], g1_d), ("b1", F32, [128, 8], b1_d),
                ("g2", F32, [128, 8], g2_d), ("b2", F32, [128, 8], b2_d),
                ("bqk", F32, [128, 16], bqk_d), ("bff1", F32, [128, 32], bff1_d),
                ("mask", BF16, [128, NQ, 256], mask_d),
            ):
                st[nm] = singles.tile(shape, dt, name=nm, tag=nm)
                nc.sync.dma_start(st[nm][:], src.ap())
            for nm, src in (("vb", bv_d), ("pb", bproj_d), ("fb2", bff2_d)):
                st[nm] = singles.tile([128, D], F32, name=nm, tag=nm)
                nc.gpsimd.dma_start(st[nm][:], bcast_row(src, D))

            if niter is None:
                _block_body(tc, nc, dram, st, stop_after)
            else:
                with tc.For_i(0, niter, 1):
                    _block_body(tc, nc, dram, st, stop_after)

    nc.compile()
    return nc


def _transpose_block(nc, tc, psum_pool, src3, dst3, nt, gain=None, bias=None,
                     split_engines=True):
    """src3: [128, nt, 1024] token-major -> dst3: [128, 8, nt*128] dim-major.

    PE transposes 4 consecutive token-blocks into one [128, 512] PSUM tile,
    then one wide copy moves it to SBUF (alternating DVE/ACT per dim-chunk to
    balance engine load). Optional per-dim gain/bias fused into the copy.
    """
    grp = 4 if nt % 4 == 0 else 2
    for tg in range(nt // grp):
        for dc in range(8):
            pt = psum_pool.tile([128, grp * 128], BF16, tag="tp", name="pt",
                                padded_shape=[128, 1024])
            for u in range(grp):
                t = tg * grp + u
                nc.tensor.transpose(
                    pt[:, u * 128:(u + 1) * 128],
                    src3[:, t, dc * 128:(dc + 1) * 128], nc._ident[:])
            dst = dst3[:, dc, tg * grp * 128:(tg + 1) * grp * 128]
            use_act = split_engines and (dc % 2 == 1)
            if gain is not None:
                if use_act:
                    nc.scalar.activation(
                        out=dst, in_=pt[:], func=AF.Identity,
                        bias=bias[:, dc:dc + 1], scale=gain[:, dc:dc + 1])
                else:
                    nc.vector.tensor_scalar(
                        out=dst, in0=pt[:],
                        scalar1=gain[:, dc:dc + 1], scalar2=bias[:, dc:dc + 1],
                        op0=ALU.mult, op1=ALU.add)
            else:
                if use_act:
                    nc.scalar.copy(out=dst, in_=pt[:])
                else:
                    nc.vector.tensor_copy(dst, pt[:])


def _block_body(tc, nc, dram, st, stop_after=None):
    xf_d, xq_d, y_d = dram["xf"], dram["xq"], dram["y"]
    eps_t = st["eps"]
    x2_d = nc.dram_tensor("x2_scratch", [TQ, D], F32, kind="Internal")

    # SBUF (LIFO per side):
    #   right: qkv { xtf { xtq } } -- xtq dies after Q, xtf after K-pair-1
    #     (mid-slot-loop), qkv after the slot loop; then xt2.
    #   left: p1/p1s/xln/wqk (prefix), slot pools, post pools, ht.
    # PSUM: p2ps(4) | pre(6)+p2psb(2) | mm(2)+sc(4)+pv(2) | post(4) | ff1(4) |
    #   ff2(8).
    qkv_p = tc.alloc_tile_pool(name="qkv", bufs=1, side="right")
    QT = qkv_p.tile([128, 8, TQ], BF16, name="QT", tag="qt")
    KT = qkv_p.tile([128, 8, T], BF16, name="KT", tag="kt")
    V = qkv_p.tile([128, NB, H, HD + 1], BF16, name="V", tag="v")
    for t in range(NB):
        nc.vector.memset(V[:, t, :, HD:HD + 1], 1.0)

    xtf_p = tc.alloc_tile_pool(name="xtf", bufs=1, side="right")
    XTF = xtf_p.tile([128, 8, T], BF16, name="XTF", tag="xtf")
    wqk_p = tc.alloc_tile_pool(name="wqk", bufs=3, side="right")
    p1 = tc.alloc_tile_pool(name="p1", bufs=3, side="right")
    p1s = tc.alloc_tile_pool(name="p1s", bufs=8, side="right")
    xln_p = tc.alloc_tile_pool(name="xln", bufs=6, side="right")
    xtq_p = tc.alloc_tile_pool(name="xtq", bufs=1, side="right")
    XTQ = xtq_p.tile([128, 8, TQ], BF16, name="XTQ", tag="xtq")

    def ln_block(nt, x_d, dst3, psum_pool):
        for tg in range(nt // 4):
            tiles = []
            for u in range(4):
                t = tg * 4 + u
                xt = p1.tile([128, D], F32, tag="xt", name="xt")
                nc.sync.dma_start(xt[:], x_d.ap()[t * 128:(t + 1) * 128, :])
                ln = xln_p.tile([128, D], BF16, tag="ln", name="ln")
                _layernorm_tile(nc, p1s, xt[:], ln[:], eps_t)
                tiles.append(ln)
            for dc in range(8):
                pt = psum_pool.tile([128, 512], BF16, tag="tp", name="pt",
                                    padded_shape=[128, 1024])
                for u in range(4):
                    nc.tensor.transpose(
                        pt[:, u * 128:(u + 1) * 128],
                        tiles[u][:, dc * 128:(dc + 1) * 128], nc._ident[:])
                dst = dst3[:, dc, tg * 512:(tg + 1) * 512]
                if dc % 2 == 1:
                    nc.scalar.activation(
                        out=dst, in_=pt[:], func=AF.Identity,
                        bias=st["b1"][:, dc:dc + 1], scale=st["g1"][:, dc:dc + 1])
                else:
                    nc.vector.tensor_scalar(
                        out=dst, in0=pt[:],
                        scalar1=st["g1"][:, dc:dc + 1], scalar2=st["b1"][:, dc:dc + 1],
                        op0=ALU.mult, op1=ALU.add)

    def make_qk(cc, pair, mm_pool, pstag="mm"):
        """Produce one 1024-token pair of QT (cc<8) or KT (cc>=8) chunk cc."""
        is_q = cc < 8
        XT = XTQ if is_q else XTF
        OUT = QT if is_q else KT
        occ = cc if is_q else cc - 8
        w = wqk_p.tile([128, 8, 128], BF16, tag="wqk", name="w")
        nc.sync.dma_start(
            w[:], dram["wqkv"].ap()[:, :, cc * 128:(cc + 1) * 128])
        ps = mm_pool.tile([128, 1024], F32, tag=pstag, name="ps")
        for h2 in range(2):
            for dc in range(8):
                nc.tensor.matmul(
                    ps[:, h2 * 512:(h2 + 1) * 512],
                    w[:, dc, :],
                    XT[:, dc, (pair * 2 + h2) * 512:(pair * 2 + h2 + 1) * 512],
                    start=(dc == 0), stop=(dc == 7),
                )
        nc.vector.tensor_scalar_add(
            out=OUT[:, occ, pair * 1024:(pair + 1) * 1024],
            in0=ps[:], scalar1=st["bqk"][:, cc:cc + 1],
        )

    # ---- prefix: LN(x_full); LN(x_q); V; K pair 0; Q ----------------------
    p2ps = tc.alloc_tile_pool(name="p2ps", bufs=4, space=PSUM)
    ln_block(NB, xf_d, XTF, p2ps)
    p2ps.release()

    pre_ps = tc.alloc_tile_pool(name="pre_ps", bufs=3, space=PSUM)
    with tc.tile_pool(name="p2psb", bufs=2, space=PSUM) as p2psb, \
         tc.tile_pool(name="wv", bufs=1) as wv_p:
        for vh in range(2):
            wv = wv_p.tile([128, 8, 512], BF16, tag="wv")
            nc.sync.dma_start(
                wv[:], dram["wqkv"].ap()[:, :, 2 * D + vh * 512:
                                         2 * D + (vh + 1) * 512])
            for t in range(NB):
                ps = pre_ps.tile([128, 512], F32, tag="mm", name="ps",
                                 padded_shape=[128, 1024])
                for dc in range(8):
                    nc.tensor.matmul(
                        ps[:], XTF[:, dc, t * 128:(t + 1) * 128],
                        wv[:, dc, :], start=(dc == 0), stop=(dc == 7),
                    )
                nc.vector.tensor_add(
                    out=V[:, t, vh * 8:(vh + 1) * 8, 0:HD],
                    in0=ps[:].rearrange("p (h d) -> p h d", h=8),
                    in1=st["vb"][:, vh * 512:(vh + 1) * 512]
                        .rearrange("p (h d) -> p h d", h=8),
                )
        # x_q layernorm overlaps the V matmuls on DVE/ACT
        ln_block(NQ, xq_d, XTQ, p2psb)
        for cc in range(8, 16):
            make_qk(cc, 0, pre_ps)
        for cc in range(8):
            make_qk(cc, 0, pre_ps)
    xtq_p.release()
    xln_p.release()
    p1s.release()
    p1.release()
    pre_ps.release()
    if stop_after == "prefix":
        wqk_p.release()
        xtf_p.release()
        qkv_p.release()
        return

    # ---- attention: slot-major, ascending causal size; K pair 1 and the
    # ---- per-slot ATT-transpose+proj tails injected into the stream -------
    # one shared 3-slot psum pool for scores + proj + K1 production: scores
    # of the next head-pair can start while the previous pair's exp drains.
    big_ps = tc.alloc_tile_pool(name="big_ps", bufs=3, space=PSUM)
    mm_ps = big_ps
    sc_ps = big_ps
    pv_ps = tc.alloc_tile_pool(name="pv_ps", bufs=2, space=PSUM)
    att_pool = tc.alloc_tile_pool(name="attsl", bufs=3)
    attt_pool = tc.alloc_tile_pool(name="atttsl", bufs=2)
    atp = tc.alloc_tile_pool(name="at", bufs=6)
    epp = tc.alloc_tile_pool(name="ep", bufs=4)
    p7 = tc.alloc_tile_pool(name="p7", bufs=2)
    wpp = tc.alloc_tile_pool(name="wproj", bufs=1)
    PW = wpp.tile([128, 8, D], BF16, name="PW", tag="pw")
    nc.sync.dma_start(PW[:], dram["wproj"].ap())

    def emit_scores_pair(hc, j, att_j):
        """Scores+exp+mask for BOTH heads of pair hc at slot j.

        The two heads' score matmuls are emitted adjacently per key block:
        their lhsT base partitions are 0 and 64, so the PE runs them on
        disjoint row-groups concurrently (K=64 each fills half the array).
        """
        nblk = NKMAX[j]
        out = []
        for hp in range(2):
            out.append((2 * hc + hp, j, [], att_j))
        for g in range((nblk + 7) // 8):
            blo = g * 8
            bhi = min(blo + 8, nblk)
            ncol = (bhi - blo) * 128
            pss = [sc_ps.tile([128, 1024], F32, tag="big", name="ps")
                   for _ in range(2)]
            for kk in range(blo, bhi):
                for hp in range(2):
                    po = hp * 64
                    nc.tensor.matmul(
                        pss[hp][:, (kk - blo) * 128:(kk - blo + 1) * 128],
                        KT[po:po + 64, hc, kk * 128:(kk + 1) * 128],
                        QT[po:po + 64, hc, j * 128:(j + 1) * 128],
                        start=True, stop=True,
                    )
            for hp in range(2):
                at = atp.tile([128, 1024], BF16, tag="at", name="at")
                nc.scalar.activation(out=at[:, 0:ncol], in_=pss[hp][:, 0:ncol],
                                     func=AF.Exp, scale=0.125)
                for kk in range(max(blo, nblk - 2), bhi):
                    mc = (kk - (nblk - 2)) * 128
                    nc.vector.tensor_mul(
                        out=at[:, (kk - blo) * 128:(kk - blo + 1) * 128],
                        in0=at[:, (kk - blo) * 128:(kk - blo + 1) * 128],
                        in1=st["mask"][:, j, mc:mc + 128],
                    )
                out[hp][2].append((blo, bhi, at))
        return out

    def emit_pv(pend):
        h, j, ats, att_j = pend
        nblk = NKMAX[j]
        pv = pv_ps.tile([128, HD + 1], F32, tag="pv", name="pv")
        for blo, bhi, at in ats:
            for kk in range(blo, bhi):
                nc.tensor.matmul(
                    pv[:],
                    at[:, (kk - blo) * 128:(kk - blo + 1) * 128],
                    V[:, kk, h, :],
                    start=(kk == 0), stop=(kk == nblk - 1),
                )
        r = epp.tile([128, 1], F32, tag="recip", name="r")
        nc.vector.reciprocal(out=r[:], in_=pv[:, HD:HD + 1])
        nc.vector.tensor_scalar_mul(
            out=att_j[:, h * HD:(h + 1) * HD],
            in0=pv[:, 0:HD], scalar1=r[:],
        )

    def slot_tail(j, att_j):
        """ATT_j -> transpose -> proj -> +bias +x_q -> X2 scratch (DRAM)."""
        attt = attt_pool.tile([128, 8, 128], BF16, tag="attt", name="attt")
        for g2 in range(2):
            pt = sc_ps.tile([128, 512], BF16, tag="big", name="pt",
                            padded_shape=[128, 2048])
            for u in range(4):
                dc = g2 * 4 + u
                nc.tensor.transpose(
                    pt[:, u * 128:(u + 1) * 128],
                    att_j[:, dc * 128:(dc + 1) * 128], nc._ident[:])
            nc.vector.tensor_copy(attt[:, g2 * 4:(g2 + 1) * 4, :], pt[:])
        ps = mm_ps.tile([128, 1024], F32, tag="big", name="ps")
        for half in range(2):
            for hcc in range(8):
                nc.tensor.matmul(
                    ps[:, half * 512:(half + 1) * 512],
                    attt[:, hcc, :],
                    PW[:, hcc, half * 512:(half + 1) * 512],
                    start=(hcc == 0), stop=(hcc == 7),
                )
        xq = p7.tile([128, D], F32, tag="xq", name="xq")
        nc.sync.dma_start(xq[:], xq_d.ap()[j * 128:(j + 1) * 128, :])
        x2t = p7.tile([128, D], F32, tag="x2t", name="x2t")
        nc.vector.tensor_add(out=x2t[:], in0=ps[:], in1=st["pb"][:])
        nc.vector.tensor_add(out=x2t[:], in0=x2t[:], in1=xq[:])
        nc.sync.dma_start(x2_d.ap()[j * 128:(j + 1) * 128, :], x2t[:])

    # production tasks injected between attention stages (every 2 stages)
    inject = [("k1", cc) for cc in range(8, 16)]
    inject_every = 2
    stage = 0
    pending = None
    done = []
    xtf_released = False
    for j in reversed(range(NQ)):  # ascending causal size: 2,4,...,16 blocks
        att_j = att_pool.tile([128, D], BF16, tag="att", name="att_j")
        for hc in range(8):
            cur2 = emit_scores_pair(hc, j, att_j)
            for cur in cur2:
                if pending is not None:
                    emit_pv(pending)
                pending = cur
            stage += 1
            if stage % inject_every == 0 and inject:
                make_qk(inject.pop(0)[1], 1, mm_ps, pstag="big")
            if hc == 0 and done:
                slot_tail(*done.pop())
            if not inject and not xtf_released:
                wqk_p.release()
                xtf_p.release()
                xtf_released = True
        done.append((j, att_j))
    emit_pv(pending)
    slot_tail(*done.pop())

    wpp.release()
    p7.release()
    epp.release()
    atp.release()
    attt_pool.release()
    att_pool.release()
    pv_ps.release()
    big_ps.release()
    qkv_p.release()
    if stop_after == "p5":
        return

    # ---- post: reload X2, LN2, transpose -> XT2 ---------------------------
    xt2_p = tc.alloc_tile_pool(name="xt2", bufs=1, side="right")
    XT2 = xt2_p.tile([128, 8, TQ], BF16, name="XT2", tag="xt2")
    post_ps = tc.alloc_tile_pool(name="post_ps", bufs=4, space=PSUM)
    with tc.tile_pool(name="pp", bufs=4) as pp, \
         tc.tile_pool(name="pps", bufs=8) as pps, \
         tc.tile_pool(name="ppl", bufs=6) as ppl:
        for tg in range(2):
            tiles = []
            for u in range(4):
                j = tg * 4 + u
                x2t = pp.tile([128, D], F32, tag="x2t", name="x2t")
                nc.sync.dma_start(x2t[:], x2_d.ap()[j * 128:(j + 1) * 128, :])
                ln = ppl.tile([128, D], BF16, tag="ln2", name="ln2")
                _layernorm_tile(nc, pps, x2t[:], ln[:], eps_t)
                tiles.append(ln)
            for dc in range(8):
                pt = post_ps.tile([128, 512], BF16, tag="tp", name="pt",
                                  padded_shape=[128, 1024])
                for u in range(4):
                    nc.tensor.transpose(
                        pt[:, u * 128:(u + 1) * 128],
                        tiles[u][:, dc * 128:(dc + 1) * 128], nc._ident[:])
                dst = XT2[:, dc, tg * 512:(tg + 1) * 512]
                if dc % 2 == 1:
                    nc.scalar.activation(
                        out=dst, in_=pt[:], func=AF.Identity,
                        bias=st["b2"][:, dc:dc + 1], scale=st["g2"][:, dc:dc + 1])
                else:
                    nc.vector.tensor_scalar(
                        out=dst, in0=pt[:],
                        scalar1=st["g2"][:, dc:dc + 1], scalar2=st["b2"][:, dc:dc + 1],
                        op0=ALU.mult, op1=ALU.add)
    post_ps.release()
    if stop_after == "p7":
        xt2_p.release()
        return

    # ---- P9: FFN ----------------------------------------------------------
    ht_p = tc.alloc_tile_pool(name="ht", bufs=1)
    HT = ht_p.tile([128, 32, TQ], BF16, name="HT", tag="ht")
    X2F = ht_p.tile([128, NQ, D], F32, name="X2F", tag="x2f")
    with tc.tile_pool(name="x2f_ld", bufs=3) as x2fp:
        for t in range(NQ):
            x2t = x2fp.tile([128, D], F32, tag="x2l", name="x2l")
            nc.sync.dma_start(x2t[:], x2_d.ap()[t * 128:(t + 1) * 128, :])
            nc.vector.tensor_add(out=X2F[:, t, :], in0=x2t[:], in1=st["fb2"][:])
    ff1_ps = tc.alloc_tile_pool(name="ff1_ps", bufs=2, space=PSUM)
    with tc.tile_pool(name="w1", bufs=4) as w1p:
        for hh in range(32):
            w1 = w1p.tile([128, 8, 128], BF16, tag="w1")
            nc.sync.dma_start(
                w1[:], dram["wff1"].ap()[:, :, hh * 128:(hh + 1) * 128])
            ps = ff1_ps.tile([128, 1024], F32, tag="mm", name="ps")
            for half in range(2):
                for dc in range(8):
                    nc.tensor.matmul(
                        ps[:, half * 512:(half + 1) * 512],
                        w1[:, dc, :],
                        XT2[:, dc, half * 512:(half + 1) * 512],
                        start=(dc == 0), stop=(dc == 7),
                    )
            nc.scalar.activation(
                out=HT[:, hh, :], in_=ps[:], func=AF.Gelu,
                bias=st["bff1"][:, hh:hh + 1], scale=1.0)
    ff1_ps.release()
    xt2_p.release()
    if stop_after == "ff1":
        ht_p.release()
        return

    with tc.tile_pool(name="w2", bufs=6) as w2p, \
         tc.tile_pool(name="yp", bufs=3) as yp, \
         tc.tile_pool(name="ff2ps", bufs=4, space=PSUM) as ff2ps:
        for tg in range(2):
            pss = [ff2ps.tile([128, 1024], F32, name="ym", tag="ym")
                   for _ in range(4)]
            for hh in range(32):
                w2 = w2p.tile([128, D], BF16, tag="w2")
                nc.sync.dma_start(w2[:], dram["wff2"].ap()[:, hh, :])
                for tt in range(4):
                    t = tg * 4 + tt
                    for half in range(2):
                        nc.tensor.matmul(
                            pss[tt][:, half * 512:(half + 1) * 512],
                            HT[:, hh, t * 128:(t + 1) * 128],
                            w2[:, half * 512:(half + 1) * 512],
                            start=(hh == 0), stop=(hh == 31),
                        )
            for tt in range(4):
                t = tg * 4 + tt
                yt = yp.tile([128, D], F32, tag="yt")
                nc.vector.tensor_add(out=yt[:], in0=pss[tt][:], in1=X2F[:, t, :])
                nc.sync.dma_start(
                    y_d.ap()[t * 128:(t + 1) * 128, :], yt[:])
    ht_p.release()


# ---------------------------------------------------------------------------
# Host wrapper
# ---------------------------------------------------------------------------

_PROG_CACHE = {}


def _get_program(niter=None):
    if niter not in _PROG_CACHE:
        _PROG_CACHE[niter] = build_program(niter)
    return _PROG_CACHE[niter]


def make_in_maps(x, ln1_g, ln1_b, qkv_w, qkv_b, proj_w, proj_b,
                 ln2_g, ln2_b, ff1_w, ff1_b, ff2_w, ff2_b):
    bf = ml_dtypes.bfloat16
    f32 = np.float32

    def pcol(v, n):  # [n*128] -> [128, n] (chunk c holds elements c*128..c*128+127)
        return np.ascontiguousarray(np.asarray(v, f32).reshape(n, 128).T)

    def dimmajor(w, nchunk, ncol):  # [nchunk*128, ncol] -> [128, nchunk, ncol]
        return np.ascontiguousarray(
            np.asarray(w, f32).reshape(nchunk, 128, ncol).transpose(1, 0, 2)
        ).astype(bf)

    qkv_b = np.asarray(qkv_b, f32)
    common = dict(
        wqkv=dimmajor(qkv_w, 8, 3 * D),
        wproj=dimmajor(proj_w, 8, D),
        wff1=dimmajor(ff1_w, 8, HID),
        wff2=dimmajor(ff2_w, 32, D),
        bqk=np.ascontiguousarray(
            np.concatenate([pcol(qkv_b[0:D], 8), pcol(qkv_b[D:2 * D], 8)], axis=1)),
        bv=qkv_b[2 * D:3 * D].copy(),
        bproj=np.asarray(proj_b, f32).copy(),
        bff1=pcol(ff1_b, 32),
        bff2=np.asarray(ff2_b, f32).copy(),
        g1=pcol(ln1_g, 8), b1=pcol(ln1_b, 8),
        g2=pcol(ln2_g, 8), b2=pcol(ln2_b, 8),
    )
    masks = [np.ascontiguousarray(_masks(p).transpose(1, 0, 2)).astype(bf)
             for p in range(2)]
    perms = [_perm(0), _perm(1)]

    x = np.asarray(x, f32)
    in_maps = []
    for c in range(N_CORES):
        b, p = c // 2, c % 2
        m = dict(common)
        m["x_full"] = np.ascontiguousarray(x[b])
        m["x_q"] = np.ascontiguousarray(x[b][perms[p]])
        m["mask"] = masks[p]
        in_maps.append(m)
    return in_maps, perms


def kernel(**inputs):
    in_maps, perms = make_in_maps(**{k: np.asarray(v) for k, v in inputs.items()})
    nc = _get_program()
    res = run_bass_kernel_spmd(nc, in_maps, list(range(N_CORES))).results
    y = np.empty((B, T, D), np.float32)
    for c in range(N_CORES):
        b, p = c // 2, c % 2
        y[b][perms[p]] = res[c]["y"]
    return y

